# revision 9
# baseline (speedup 1.0000x reference)
"""Trainium2 Bass kernel for nn_Decoder_63866163691780 (6-block transformer decoder).

Sharding (8 NeuronCores, one chip):
  - Attention is head-parallel: core c owns heads {2c, 2c+1} (= d_model columns
    [128c, 128c+128)) for all 4 batch elements.
  - LayerNorm / residual / FFN / input projections are token-parallel: core c owns
    rows [512c, 512c+512) of the flattened [4096, 1024] activation.
  - The two shardings are bridged with 8-rank AllToAll collectives (4 per block):
    packed QKV1, attention-out a1, cross-query q2, attention-out a2.

Layout conventions inside a core:
  - Activations that feed matmuls live as xT [D on partitions, tokens free].
  - Attention scores are computed transposed: scoresT [k-tokens on partitions,
    q-tokens free]; softmax normalization comes from a ones column appended to V
    (probs @ [V|1] yields numerator and denominator together), and the PV matmul
    (lhsT=expT tile, rhs=V) directly produces [q, dk] output — no transposes in
    the attention path.
  - Causal masking: strictly-upper score tiles are skipped; diagonal tiles are
    multiplied by one of 4 precomputed [128, 512] patterns.
  - Matmul operands are bf16 (fp32 accumulation in PSUM); LayerNorm statistics,
    residual stream, and softmax normalization are fp32.
"""
import numpy as np
import ml_dtypes

import concourse.bass as bass
import concourse.mybir as mybir
import concourse.bacc as bacc
import concourse.tile as tile
from concourse import bass_utils
from concourse.masks import make_identity

F32 = mybir.dt.float32

B, S, D, DF, H, DK = 4, 1024, 1024, 4096, 16, 64
L = 6
NC = 8            # cores
TOK = 512         # tokens per core (token sharding)
HD = 128          # head-group dims per core (2 heads x 64)
ND = D // 128     # 8 d-tiles
NF = DF // 128    # 32 ffn tiles
EPS = 1e-5
RG = [list(range(NC))]

# matmul operand dtype knob: mybir.dt.bfloat16 | float32r | float32
DT_MM = mybir.dt.bfloat16


def _np_dt(dt_mm):
    return ml_dtypes.bfloat16 if dt_mm == mybir.dt.bfloat16 else np.float32


def build_nc(dt_mm=DT_MM, n_blocks=L, use_bias=False, ln_affine=False):
    """Build the single-core SPMD program (same program on all 8 cores; data differs)."""
    dt = dt_mm
    nc = bacc.Bacc("TRN2", target_bir_lowering=False, debug=False, num_devices=NC)

    # ---------------- DRAM I/O ----------------
    x0T = nc.dram_tensor("x0T", [D, TOK], dt, kind="ExternalInput")
    x0s = nc.dram_tensor("x0s", [TOK, D], F32, kind="ExternalInput")
    encT = nc.dram_tensor("encT", [D, B * S], dt, kind="ExternalInput")
    wq1 = nc.dram_tensor("wq1", [n_blocks, D, D], dt, kind="ExternalInput")
    wk1 = nc.dram_tensor("wk1", [n_blocks, D, D], dt, kind="ExternalInput")
    wv1 = nc.dram_tensor("wv1", [n_blocks, D, D], dt, kind="ExternalInput")
    wq2 = nc.dram_tensor("wq2", [n_blocks, D, D], dt, kind="ExternalInput")
    wk2s = nc.dram_tensor("wk2s", [n_blocks, D, HD], dt, kind="ExternalInput")
    wv2s = nc.dram_tensor("wv2s", [n_blocks, D, HD], dt, kind="ExternalInput")
    wf1 = nc.dram_tensor("wf1", [n_blocks, D, DF], dt, kind="ExternalInput")
    wf2 = nc.dram_tensor("wf2", [n_blocks, DF, D], dt, kind="ExternalInput")
    cmask = nc.dram_tensor("cmask", [4, 128, 512], dt, kind="ExternalInput")
    if use_bias:
        bq1 = nc.dram_tensor("bq1", [n_blocks, D], F32, kind="ExternalInput")
        bk1 = nc.dram_tensor("bk1", [n_blocks, D], F32, kind="ExternalInput")
        bv1 = nc.dram_tensor("bv1", [n_blocks, D], F32, kind="ExternalInput")
        bq2 = nc.dram_tensor("bq2", [n_blocks, D], F32, kind="ExternalInput")
        bk2s = nc.dram_tensor("bk2s", [n_blocks, HD], F32, kind="ExternalInput")
        bv2s = nc.dram_tensor("bv2s", [n_blocks, HD], F32, kind="ExternalInput")
        bf1 = nc.dram_tensor("bf1", [n_blocks, DF], F32, kind="ExternalInput")
        bf2 = nc.dram_tensor("bf2", [n_blocks, D], F32, kind="ExternalInput")
    if ln_affine:
        lng = [nc.dram_tensor(f"g{j}", [n_blocks, D], F32, kind="ExternalInput") for j in (1, 2, 3)]
        lnb = [nc.dram_tensor(f"be{j}", [n_blocks, D], F32, kind="ExternalInput") for j in (1, 2, 3)]
    out = nc.dram_tensor("out", [TOK, D], F32, kind="ExternalOutput")

    import contextlib
    with tile.TileContext(nc) as tc, contextlib.ExitStack() as ctx:
        dram = ctx.enter_context(tc.tile_pool(name="dram", bufs=1, space="DRAM"))
        wpool = ctx.enter_context(tc.tile_pool(name="wpool", bufs=1))
        xtp = ctx.enter_context(tc.tile_pool(name="xtp", bufs=1))
        resid = ctx.enter_context(tc.tile_pool(name="resid", bufs=1))
        attn = ctx.enter_context(tc.tile_pool(name="attn", bufs=1))
        expp = ctx.enter_context(tc.tile_pool(name="expp", bufs=1))
        small = ctx.enter_context(tc.tile_pool(name="small", bufs=1))
        hpool = ctx.enter_context(tc.tile_pool(name="hpool", bufs=1))
        singles = ctx.enter_context(tc.tile_pool(name="singles", bufs=1))
        drains = ctx.enter_context(tc.tile_pool(name="drains", bufs=1))
        psum = ctx.enter_context(tc.tile_pool(name="psum", bufs=1, space="PSUM"))

        def pj_tile():
            t = psum.tile([128, 512], F32, tag="pj", bufs=2, name="pj")
            return t

        # ---------------- one-time setup ----------------
        ident = singles.tile([128, 128], F32, tag="ident")
        make_identity(nc, ident)
        masks = singles.tile([128, 4, 512], dt, tag="masks")
        nc.sync.dma_start(masks[:], cmask.ap().rearrange("r p f -> p r f"))
        eps_t = singles.tile([128, 1], F32, tag="eps")
        nc.vector.memset(eps_t[:], EPS)

        def bias_row(dram_t, i, n, tag):
            """[n] slice of dram_t[i] -> SBUF [128, n//128] (partition = index % 128)."""
            t = small.tile([128, n // 128], F32, tag=tag, bufs=2, name=tag)
            nc.gpsimd.dma_start(t[:], dram_t[i].rearrange("(a p) -> p a", p=128))
            return t

        def bias_bcast(dram_t, i, n, tag):
            """[n] slice -> SBUF [128, n] broadcast along partitions."""
            t = small.tile([128, n], F32, tag=tag, bufs=2, name=tag)
            src = dram_t[i]
            bc = bass.AP(tensor=src.tensor, offset=src.offset, ap=[[0, 128]] + list(src.ap))
            nc.gpsimd.dma_start(t[:], bc)
            return t

        # persistent state: x shard (fp32) and its transpose (dt)
        x_sh = resid.tile([128, 4, D], F32, tag="xsh", bufs=2)
        nc.sync.dma_start(x_sh[:], x0s.ap().rearrange("(a p) d -> p a d", p=128))
        xT = xtp.tile([128, ND, TOK], dt, tag="xT", bufs=2)
        nc.sync.dma_start(xT[:], x0T.ap().rearrange("(a p) t -> p a t", p=128))

        def transpose_to(dst_xT, src_tiles):
            """src [128tok, 4, D] fp32 -> dst [128d, ND, TOK] dt via PE transpose."""
            for tt in range(4):
                for d in range(ND):
                    pt = psum.tile([128, 128], F32, tag="pj", bufs=2, name="pt")
                    nc.tensor.transpose(pt[:], src_tiles[:, tt, d * 128:(d + 1) * 128], ident[:])
                    nc.vector.tensor_copy(dst_xT[:, d, tt * 128:(tt + 1) * 128], pt[:])

        def layer_norm(xf, dst, i, which):
            """xf: [128, 4, D] fp32 (post-residual); dst: [128, 4, D] fp32 normalized."""
            for tt in range(4):
                stats = small.tile([128, 2, 6], F32, tag="lnstats", bufs=4)
                nc.vector.bn_stats(stats[:, 0, :], xf[:, tt, 0:512])
                nc.vector.bn_stats(stats[:, 1, :], xf[:, tt, 512:1024])
                mv = small.tile([128, 2], F32, tag="lnmv", bufs=4)
                nc.vector.bn_aggr(mv[:], stats[:])
                rs = small.tile([128, 1], F32, tag="lnrs", bufs=4)
                nc.scalar.activation(rs[:], mv[:, 1:2], mybir.ActivationFunctionType.Sqrt, bias=eps_t[:])
                nc.vector.reciprocal(rs[:], rs[:])
                nc.vector.tensor_scalar(
                    out=dst[:, tt, :], in0=xf[:, tt, :], scalar1=mv[:, 0:1], scalar2=rs[:],
                    op0=mybir.AluOpType.subtract, op1=mybir.AluOpType.mult)
                if ln_affine:
                    g_bc = bias_bcast(lng[which], i, D, f"lng{which}")
                    b_bc = bias_bcast(lnb[which], i, D, f"lnb{which}")
                    nc.vector.tensor_mul(dst[:, tt, :], dst[:, tt, :], g_bc[:])
                    nc.vector.tensor_add(dst[:, tt, :], dst[:, tt, :], b_bc[:])

        def attention_half(qT_b, kT_b, vaug, a2a_dst, b, causal):
            """One batch, both owned heads. qT_b/kT_b: [128, 2, 512] SBUF (dims x tok);
            vaug: 8 tiles [128, 130]; writes normalized [q,128] tiles into a2a_dst."""
            for ch in range(2):
                kt_hi = (4 * ch + 4) if causal else 8
                exps = []
                for kt in range(kt_hi):
                    pair = []
                    for h in range(2):
                        sc = psum.tile([128, 512], F32, tag="sc", bufs=2, name="sc")
                        nc.tensor.matmul(
                            sc[:], kT_b[64 * h:64 * h + 64, kt // 4, (kt % 4) * 128:(kt % 4) * 128 + 128],
                            qT_b[64 * h:64 * h + 64, ch, :], start=True, stop=True)
                        et = expp.tile([128, 512], dt, tag="expT", bufs=18, name="et")
                        nc.scalar.activation(et[:], sc[:], mybir.ActivationFunctionType.Exp, scale=0.125)
                        if causal and kt - 4 * ch >= 0:
                            nc.vector.tensor_mul(et[:], et[:], masks[:, kt - 4 * ch, :])
                        pair.append(et)
                    exps.append(pair)
                for qt in range(4):
                    qt_g = ch * 4 + qt
                    kt_n = (qt_g + 1) if causal else 8
                    pva = psum.tile([128, 130], F32, tag="pva", bufs=2, name="pva")
                    for h in range(2):
                        for kt in range(kt_n):
                            nc.tensor.matmul(pva[:, 65 * h:65 * h + 65],
                                             exps[kt][h][:, qt * 128:(qt + 1) * 128],
                                             vaug[kt][:, 65 * h:65 * h + 65],
                                             start=(kt == 0), stop=(kt == kt_n - 1))
                    rcp = small.tile([128, 2], F32, tag="rcp", bufs=4, name="rcp")
                    nc.vector.reciprocal(rcp[:], pva[:, 64:130:65])
                    a1t = drains.tile([128, 128], dt, tag="a1t", bufs=4, name="a1t")
                    nc.vector.tensor_scalar_mul(a1t[:, 0:64], pva[:, 0:64], rcp[:, 0:1])
                    nc.vector.tensor_scalar_mul(a1t[:, 64:128], pva[:, 65:129], rcp[:, 1:2])
                    jd = 2 * b + qt_g // 4
                    off = (qt_g * 128) % 512
                    nc.sync.dma_start(a2a_dst[jd, off:off + 128, :], a1t[:])

        # ---------------- per-block ----------------
        for i in range(n_blocks):
            # ===== Phase A: token-sharded QKV1 + packed A2A =====
            a2a_qkv_i = dram.tile([NC, 3, 128 * TOK], dt, tag="qkvi", bufs=2)
            a2a_qkv_o = dram.tile([NC, 3, 128 * TOK], dt, tag="qkvo", bufs=2)
            bq1_s = bias_row(bq1, i, D, "bq1") if use_bias else None
            bk1_s = bias_row(bk1, i, D, "bk1") if use_bias else None
            for (wdr, slot, b_s) in ((wq1, 0, bq1_s), (wk1, 1, bk1_s)):
                for ft in range(ND):
                    wt = wpool.tile([128, ND, 128], dt, tag="wqk", bufs=3, name="wt")
                    nc.sync.dma_start(wt[:], wdr[i, :, ft * 128:(ft + 1) * 128].rearrange("(a p) f -> p a f", p=128))
                    pj = pj_tile()
                    for d in range(ND):
                        nc.tensor.matmul(pj[:], wt[:, d, :], xT[:, d, :], start=(d == 0), stop=(d == ND - 1))
                    qs = drains.tile([128, TOK], dt, tag="dr2k", bufs=3, name="qs")
                    if use_bias:
                        nc.vector.tensor_scalar_add(qs[:], pj[:], b_s[:, ft:ft + 1])
                    else:
                        nc.vector.tensor_copy(qs[:], pj[:])
                    nc.sync.dma_start(a2a_qkv_i[ft, slot].rearrange("(p f) -> p f", p=128), qs[:])
            # V1 in [tok, D] layout (stationary xT tiles, moving wv1)
            bv1_bc = bias_bcast(bv1, i, D, "bv1") if use_bias else None
            for fc in range(2):
                wvt = wpool.tile([128, ND, 512], dt, tag="w16", bufs=2, name="wvt")
                nc.sync.dma_start(wvt[:], wv1[i, :, fc * 512:(fc + 1) * 512].rearrange("(a p) f -> p a f", p=128))
                for tt in range(4):
                    pj = pj_tile()
                    for d in range(ND):
                        nc.tensor.matmul(pj[:], xT[:, d, tt * 128:(tt + 1) * 128], wvt[:, d, :],
                                         start=(d == 0), stop=(d == ND - 1))
                    vs = drains.tile([128, 512], dt, tag="dr2k", bufs=3, name="vs")
                    if use_bias:
                        nc.vector.tensor_add(vs[:], pj[:], bv1_bc[:, fc * 512:(fc + 1) * 512])
                    else:
                        nc.vector.tensor_copy(vs[:], pj[:])
                    for j in range(4):
                        jd = fc * 4 + j
                        nc.sync.dma_start(
                            a2a_qkv_i[jd, 2].rearrange("(t f) -> t f", f=128)[tt * 128:(tt + 1) * 128, :],
                            vs[:, j * 128:(j + 1) * 128])
            nc.gpsimd.collective_compute("AllToAll", mybir.AluOpType.bypass, replica_groups=RG,
                                         ins=[a2a_qkv_i.opt()], outs=[a2a_qkv_o.opt()])

            # ===== Phase B: self-attention =====
            a2a_a1_i = dram.tile([NC, TOK, 128], dt, tag="a1i", bufs=2)
            a2a_a1_o = dram.tile([NC, TOK, 128], dt, tag="a1o", bufs=2)
            for b in range(4):
                qT_b = attn.tile([128, 2, 512], dt, tag="qkb", bufs=6, name="qT_b")
                kT_b = attn.tile([128, 2, 512], dt, tag="qkb", bufs=6, name="kT_b")
                for jj in range(2):
                    nc.sync.dma_start(qT_b[:, jj, :], a2a_qkv_o[2 * b + jj, 0].rearrange("(p f) -> p f", p=128))
                    nc.sync.dma_start(kT_b[:, jj, :], a2a_qkv_o[2 * b + jj, 1].rearrange("(p f) -> p f", p=128))
                vaug = []
                for kt in range(8):
                    jd = 2 * b + kt // 4
                    off = (kt % 4) * 128
                    va = attn.tile([128, 130], dt, tag="vaug", bufs=10, name="va")
                    src = a2a_qkv_o[jd, 2].rearrange("(t f) -> t f", f=128)
                    nc.sync.dma_start(va[:, 0:64], src[off:off + 128, 0:64])
                    nc.sync.dma_start(va[:, 65:129], src[off:off + 128, 64:128])
                    nc.vector.memset(va[:, 64:65], 1.0)
                    nc.vector.memset(va[:, 129:130], 1.0)
                    vaug.append(va)
                attention_half(qT_b, kT_b, vaug, a2a_a1_i, b, causal=True)
            nc.gpsimd.collective_compute("AllToAll", mybir.AluOpType.bypass, replica_groups=RG,
                                         ins=[a2a_a1_i.opt()], outs=[a2a_a1_o.opt()])

            # ===== Phase C: residual + LN1 + transpose =====
            x1_sh = resid.tile([128, 4, D], F32, tag="xsh", bufs=2)
            for tt in range(4):
                a1r = drains.tile([128, NC, 128], dt, tag="arecv", bufs=2, name="a1r")
                nc.sync.dma_start(a1r[:], a2a_a1_o[:, tt * 128:(tt + 1) * 128, :].rearrange("j t f -> t j f"))
                nc.vector.tensor_add(x_sh[:, tt, :], x_sh[:, tt, :],
                                     a1r.rearrange("p j f -> p (j f)"))
            layer_norm(x_sh, x1_sh, i, 0)
            x1T = xtp.tile([128, ND, TOK], dt, tag="xT", bufs=2)
            transpose_to(x1T, x1_sh)

            # ===== Phase D: cross attention =====
            a2a_q2_i = dram.tile([NC, 128, TOK], dt, tag="q2i", bufs=2)
            a2a_q2_o = dram.tile([NC, 128, TOK], dt, tag="q2o", bufs=2)
            bq2_s = bias_row(bq2, i, D, "bq2") if use_bias else None
            for ft in range(ND):
                wt = wpool.tile([128, ND, 128], dt, tag="wqk", bufs=3, name="wt2")
                nc.sync.dma_start(wt[:], wq2[i, :, ft * 128:(ft + 1) * 128].rearrange("(a p) f -> p a f", p=128))
                pj = pj_tile()
                for d in range(ND):
                    nc.tensor.matmul(pj[:], wt[:, d, :], x1T[:, d, :], start=(d == 0), stop=(d == ND - 1))
                qs = drains.tile([128, TOK], dt, tag="dr2k", bufs=3, name="q2s")
                if use_bias:
                    nc.vector.tensor_scalar_add(qs[:], pj[:], bq2_s[:, ft:ft + 1])
                else:
                    nc.vector.tensor_copy(qs[:], pj[:])
                nc.sync.dma_start(a2a_q2_i[ft], qs[:])
            nc.gpsimd.collective_compute("AllToAll", mybir.AluOpType.bypass, replica_groups=RG,
                                         ins=[a2a_q2_i.opt()], outs=[a2a_q2_o.opt()])

            a2a_a2_i = dram.tile([NC, TOK, 128], dt, tag="a2i", bufs=2)
            a2a_a2_o = dram.tile([NC, TOK, 128], dt, tag="a2o", bufs=2)
            bk2_s = bias_row(bk2s, i, HD, "bk2") if use_bias else None
            bv2_bc = bias_bcast(bv2s, i, HD, "bv2") if use_bias else None
            wk2t = wpool.tile([128, ND, 128], dt, tag="wkv2", bufs=2, name="wk2t")
            nc.sync.dma_start(wk2t[:], wk2s[i].rearrange("(a p) f -> p a f", p=128))
            wv2t = wpool.tile([128, ND, 128], dt, tag="wkv2", bufs=2, name="wv2t")
            nc.sync.dma_start(wv2t[:], wv2s[i].rearrange("(a p) f -> p a f", p=128))
            for b in range(4):
                k2T = attn.tile([128, 2, 512], dt, tag="qkb", bufs=6, name="k2T")
                vaug2 = []
                for ch in range(2):
                    enc_b = attn.tile([128, ND, 512], dt, tag="encb", bufs=2, name="enc_b")
                    nc.sync.dma_start(enc_b[:], encT[:, b * S + ch * 512:b * S + (ch + 1) * 512]
                                      .rearrange("(a p) t -> p a t", p=128))
                    pj = pj_tile()
                    for d in range(ND):
                        nc.tensor.matmul(pj[:], wk2t[:, d, :], enc_b[:, d, :],
                                         start=(d == 0), stop=(d == ND - 1))
                    if use_bias:
                        nc.vector.tensor_scalar_add(k2T[:, ch, :], pj[:], bk2_s[:, 0:1])
                    else:
                        nc.vector.tensor_copy(k2T[:, ch, :], pj[:])
                    for k4 in range(4):
                        pj2 = psum.tile([128, 128], F32, tag="pj", bufs=2, name="pj2")
                        for d in range(ND):
                            nc.tensor.matmul(pj2[:], enc_b[:, d, k4 * 128:(k4 + 1) * 128], wv2t[:, d, :],
                                             start=(d == 0), stop=(d == ND - 1))
                        va = attn.tile([128, 130], dt, tag="vaug", bufs=10, name="va2")
                        if use_bias:
                            nc.vector.tensor_add(va[:, 0:64], pj2[:, 0:64], bv2_bc[:, 0:64])
                            nc.vector.tensor_add(va[:, 65:129], pj2[:, 64:128], bv2_bc[:, 64:128])
                        else:
                            nc.vector.tensor_copy(va[:, 0:64], pj2[:, 0:64])
                            nc.vector.tensor_copy(va[:, 65:129], pj2[:, 64:128])
                        nc.vector.memset(va[:, 64:65], 1.0)
                        nc.vector.memset(va[:, 129:130], 1.0)
                        vaug2.append(va)
                q2T_b = attn.tile([128, 2, 512], dt, tag="qkb", bufs=6, name="q2T_b")
                for jj in range(2):
                    nc.sync.dma_start(q2T_b[:, jj, :], a2a_q2_o[2 * b + jj])
                attention_half(q2T_b, k2T, vaug2, a2a_a2_i, b, causal=False)
            nc.gpsimd.collective_compute("AllToAll", mybir.AluOpType.bypass, replica_groups=RG,
                                         ins=[a2a_a2_i.opt()], outs=[a2a_a2_o.opt()])

            # residual + LN2 + transpose
            x2_sh = resid.tile([128, 4, D], F32, tag="xsh", bufs=2)
            for tt in range(4):
                a2r = drains.tile([128, NC, 128], dt, tag="arecv", bufs=2, name="a2r")
                nc.sync.dma_start(a2r[:], a2a_a2_o[:, tt * 128:(tt + 1) * 128, :].rearrange("j t f -> t j f"))
                nc.vector.tensor_add(x1_sh[:, tt, :], x1_sh[:, tt, :],
                                     a2r.rearrange("p j f -> p (j f)"))
            layer_norm(x1_sh, x2_sh, i, 1)
            x2T = xtp.tile([128, ND, TOK], dt, tag="xT", bufs=2)
            transpose_to(x2T, x2_sh)

            # ===== Phase E: FFN (token-local) =====
            bf1_s = bias_row(bf1, i, DF, "bf1") if use_bias else None
            bf2_bc = bias_bcast(bf2, i, D, "bf2") if use_bias else None
            h_s = hpool.tile([128, NF, TOK], dt, tag="hs", bufs=1)
            for fc in range(8):
                w1t = wpool.tile([128, ND, 512], dt, tag="w16", bufs=2, name="w1t")
                nc.sync.dma_start(w1t[:], wf1[i, :, fc * 512:(fc + 1) * 512].rearrange("(a p) f -> p a f", p=128))
                for f4 in range(4):
                    ft = fc * 4 + f4
                    pj = pj_tile()
                    for d in range(ND):
                        nc.tensor.matmul(pj[:], w1t[:, d, f4 * 128:(f4 + 1) * 128], x2T[:, d, :],
                                         start=(d == 0), stop=(d == ND - 1))
                    if use_bias:
                        nc.scalar.activation(h_s[:, ft, :], pj[:], mybir.ActivationFunctionType.Relu,
                                             bias=bf1_s[:, ft:ft + 1])
                    else:
                        nc.scalar.activation(h_s[:, ft, :], pj[:], mybir.ActivationFunctionType.Relu)
            # FFN2: out [tok, D]; stationary h tiles, moving wf2 row-slices (re-read per tok-pair)
            for tp_ in range(2):
                for dc in range(2):
                    pjs = [psum.tile([128, 512], F32, tag="pf2", bufs=2, name=f"pf2_{tp_}_{dc}_{u}")
                           for u in range(2)]
                    for df in range(NF):
                        w2t = wpool.tile([128, 512], dt, tag="w2", bufs=3, name="w2t")
                        nc.sync.dma_start(w2t[:], wf2[i, df * 128:(df + 1) * 128, dc * 512:(dc + 1) * 512])
                        for u in range(2):
                            tt = tp_ * 2 + u
                            nc.tensor.matmul(pjs[u][:], h_s[:, df, tt * 128:(tt + 1) * 128], w2t[:],
                                             start=(df == 0), stop=(df == NF - 1))
                    for u in range(2):
                        tt = tp_ * 2 + u
                        if use_bias:
                            nc.vector.tensor_add(pjs[u][:], pjs[u][:], bf2_bc[:, dc * 512:(dc + 1) * 512])
                        nc.vector.tensor_add(x2_sh[:, tt, dc * 512:(dc + 1) * 512],
                                             x2_sh[:, tt, dc * 512:(dc + 1) * 512], pjs[u][:])
            x3_sh = resid.tile([128, 4, D], F32, tag="xsh", bufs=2)
            layer_norm(x2_sh, x3_sh, i, 2)
            if i < n_blocks - 1:
                xT = xtp.tile([128, ND, TOK], dt, tag="xT", bufs=2)
                transpose_to(xT, x3_sh)
                x_sh = x3_sh
            else:
                nc.sync.dma_start(out.ap().rearrange("(a p) d -> p a d", p=128), x3_sh[:])
    nc.compile()
    return nc


# ---------------- host side ----------------
_CACHE = {}


def _pack_inputs(encoder_output, x, params, dt_mm, n_blocks, use_bias, ln_affine):
    np_dt = _np_dt(dt_mm)
    xf = np.asarray(x, np.float32).reshape(B * S, D)
    enc = np.asarray(encoder_output, np.float32).reshape(B * S, D)
    p = {k: np.asarray(v, np.float32) for k, v in params.items()}

    cm = np.zeros((4, 128, 512), np.float32)
    for r in range(4):
        for pp_ in range(128):
            cm[r, pp_, 128 * r + pp_:] = 1.0
    common = {
        "encT": np.ascontiguousarray(enc.T).astype(np_dt),
        "wq1": p["wq1"][:n_blocks].astype(np_dt), "wk1": p["wk1"][:n_blocks].astype(np_dt),
        "wv1": p["wv1"][:n_blocks].astype(np_dt), "wq2": p["wq2"][:n_blocks].astype(np_dt),
        "wf1": p["wf1"][:n_blocks].astype(np_dt), "wf2": p["wf2"][:n_blocks].astype(np_dt),
        "cmask": cm.astype(np_dt),
    }
    if use_bias:
        common.update({
            "bq1": p["bq1"][:n_blocks], "bk1": p["bk1"][:n_blocks], "bv1": p["bv1"][:n_blocks],
            "bq2": p["bq2"][:n_blocks], "bf1": p["bf1"][:n_blocks], "bf2": p["bf2"][:n_blocks],
        })
    if ln_affine:
        common.update({
            "g1": p["g1"][:n_blocks], "be1": p["be1"][:n_blocks],
            "g2": p["g2"][:n_blocks], "be2": p["be2"][:n_blocks],
            "g3": p["g3"][:n_blocks], "be3": p["be3"][:n_blocks],
        })
    in_maps = []
    for c in range(NC):
        m = dict(common)
        sh = xf[c * TOK:(c + 1) * TOK]
        m["x0T"] = np.ascontiguousarray(sh.T).astype(np_dt)
        m["x0s"] = np.ascontiguousarray(sh)
        m["wk2s"] = np.ascontiguousarray(p["wk2"][:n_blocks, :, 128 * c:128 * c + 128]).astype(np_dt)
        m["wv2s"] = np.ascontiguousarray(p["wv2"][:n_blocks, :, 128 * c:128 * c + 128]).astype(np_dt)
        if use_bias:
            m["bk2s"] = np.ascontiguousarray(p["bk2"][:n_blocks, 128 * c:128 * c + 128])
            m["bv2s"] = np.ascontiguousarray(p["bv2"][:n_blocks, 128 * c:128 * c + 128])
        in_maps.append(m)
    return in_maps


def _needs_bias(params):
    return any(np.abs(np.asarray(params[k])).max() > 0
               for k in ("bq1", "bk1", "bv1", "bq2", "bk2", "bv2", "bf1", "bf2"))


def _needs_affine(params):
    return any(np.abs(np.asarray(params[f"g{j}"]) - 1.0).max() > 0 or
               np.abs(np.asarray(params[f"be{j}"])).max() > 0 for j in (1, 2, 3))


def get_nc(dt_mm=DT_MM, n_blocks=L, use_bias=False, ln_affine=False):
    key = (str(dt_mm), n_blocks, use_bias, ln_affine)
    if key not in _CACHE:
        _CACHE[key] = build_nc(dt_mm, n_blocks, use_bias, ln_affine)
    return _CACHE[key]


def kernel(encoder_output, x, params):
    use_bias = _needs_bias(params)
    ln_affine = _needs_affine(params)
    nc = get_nc(DT_MM, L, use_bias, ln_affine)
    in_maps = _pack_inputs(encoder_output, x, params, DT_MM, L, use_bias, ln_affine)
    res = bass_utils.run_bass_kernel_spmd(nc, in_maps, core_ids=list(range(NC)))
    shards = [res.results[c]["out"] for c in range(NC)]
    return np.concatenate(shards, axis=0).reshape(B, S, D).astype(np.float32)
